# revision 1
# baseline (speedup 1.0000x reference)
"""BitwiseTasNet Trainium2 kernel.

Full (unsharded) inputs in, full output out. Internally: data-parallel over
batch x time across 8 NeuronCores (4 time-shards per batch item) with halo
margins so no inter-core communication is needed. All matmuls run in fp32r
(full bf16-rate on PE, ~12 mantissa bits). PReLU+BatchNorm folds into a
single ScalarE Prelu eviction with per-channel scale/bias; the dilated
depthwise conv runs as 2 diagonal matmuls (outer taps) accumulating in PSUM
plus a fused DVE scalar_tensor_tensor for the center tap.
"""
import sys

sys.path.insert(0, "/opt/trn_rl_repo")

import numpy as np

import concourse.bass as bass
import concourse.mybir as mybir
import concourse.tile as tile
from concourse.bass_utils import run_bass_kernel_spmd

# Problem constants (hardcoded per contest rules).
B, T, E, D, BL, L, KT, FK, STR = 2, 64000, 256, 512, 2, 6, 3, 20, 10
EPS = 1e-5
TC = (T + 2 * FK - FK) // STR + 1  # 6403 encoder output cols
NCORES, QP = 8, 4  # 4 time-shards per batch item
NI = 1601          # interior cols per core (ceil(6403/4))
MARG = 128         # halo margin (2*63 receptive field + 2 for decoder)
NE = NI + 2 * MARG # 1857 computed cols (block 0 / encoder / decoder)
DOFF = 32          # side strip for dconv tap overhang (max dilation)
BW = 1984          # activation buffer width
XW_LEN = 19240
NL = BL * L
PCOLS_PER_LAYER = 40
NPCOL = NL * PCOLS_PER_LAYER + 8

# PSUM groups are half-width (2 banks, 4 slots) for pipeline depth. Each
# half: matmul segments (psum_off, data_off, width) and an eviction run.
# fp32r matmuls need 128-multiple widths at bank-aligned psum offsets.
# Block 1 only needs cols [63, 1794) (output valid on [126, 1731)).
HALVES0 = [
    dict(segs=[(0, 0, 512), (512, 512, 512)], ev=(0, 0, 1024)),
    dict(segs=[(0, 1024, 512), (512, 1536, 384)], ev=(0, 1024, 833)),
]
HALVES1 = [
    dict(segs=[(0, 63, 512), (512, 575, 512)], ev=(0, 63, 1024)),
    dict(segs=[(0, 1087, 512), (512, 1599, 256)], ev=(0, 1087, 707)),
]
NEW = 1920  # encoder window width (block-0 matmuls span [0, 1920))

F32 = mybir.dt.float32
F32R = mybir.dt.float32r
AF = mybir.ActivationFunctionType
OP = mybir.AluOpType

_built = None  # cached (module is data-independent)


def _split_multi_waits(nc, max_waits=1):
    """This walrus build accepts only one sync-wait command per instruction;
    hoist extras into standalone NoOps on the same engine just before it."""
    for fn in nc.m.functions:
        for blk in fn.blocks:
            new_insts, ctr = [], 0
            for inst in blk.instructions:
                si = inst.sync_info
                if si is not None and len(si.on_wait) > max_waits:
                    extra = si.on_wait[:-max_waits]
                    si.on_wait = si.on_wait[-max_waits:]
                    for w in extra:
                        ctr += 1
                        new_insts.append(mybir.InstNoOp(
                            name=f"{inst.name}_hw{ctr}",
                            engine=inst.engine,
                            sync_info=mybir.SyncInfo(on_wait=[w], on_update=[]),
                            bass_nofuse=True,
                        ))
                new_insts.append(inst)
            blk.instructions = new_insts


def build(loop_k=None):
    """Build the (data-independent) bass module for one core."""
    nc = bass.Bass()

    win_d = nc.dram_tensor("win", [FK, NEW], F32R, kind="ExternalInput")
    eye_d = nc.dram_tensor("eye", [128, 128], F32R, kind="ExternalInput")
    mkl_d = nc.dram_tensor("maskL", [128, 64], F32R, kind="ExternalInput")
    mkr_d = nc.dram_tensor("maskR", [128, 64], F32R, kind="ExternalInput")
    par_d = nc.dram_tensor("params", [128, NPCOL], F32, kind="ExternalInput")
    encT_d = nc.dram_tensor("encT", [FK, E], F32R, kind="ExternalInput")
    decT_d = nc.dram_tensor("decT", [128, 2, 20], F32R, kind="ExternalInput")
    w1T_d = nc.dram_tensor("w1T", [NL, 128, 2, D], F32R, kind="ExternalInput")
    w2T_d = nc.dram_tensor("w2T", [NL, 128, 4, E], F32R, kind="ExternalInput")
    y1_d = nc.dram_tensor("y1", [10, NI], F32, kind="ExternalOutput")
    y2_d = nc.dram_tensor("y2", [10, NI], F32, kind="ExternalOutput")

    with tile.TileContext(nc) as tc:
        with (
            tc.tile_pool(name="per", bufs=1) as per,
            tc.tile_pool(name="lw", bufs=3) as lw,
            tc.tile_pool(name="ps", bufs=4, space="PSUM") as psp,
        ):
            # ---- persistent tiles ----
            eye = per.tile([128, 128], F32R)
            mkl = per.tile([128, 64], F32R)
            mkr = per.tile([128, 64], F32R)
            par = per.tile([128, NPCOL], F32)
            encT = per.tile([FK, E], F32R)
            decT = per.tile([128, 2, 20], F32R)
            win = per.tile([FK, NEW], F32R)
            HI0 = per.tile([128, 2, BW], F32R)  # enc / block0 input (preserved)
            HI1 = per.tile([128, 2, BW], F32R)  # block1 input
            hP = per.tile([128, 2, BW], F32R)   # intra-block h scratch
            hF = per.tile([128, 2, BW], F32R)   # final h
            p = per.tile([128, 4, BW], F32R)    # prelu1 out (dconv input)
            v = per.tile([128, 4, BW], F32R)    # prelu2 out (conv2 input)
            warm = per.tile([128, 1], F32)

            nc.scalar.dma_start(encT[:], encT_d[:])
            nc.sync.dma_start(win[:, 0:512], win_d[:, 0:512])
            nc.sync.dma_start(win[:, 512:1024], win_d[:, 512:1024])
            nc.sync.dma_start(win[:, 1024:1536], win_d[:, 1024:1536])
            nc.sync.dma_start(win[:, 1536:NEW], win_d[:, 1536:NEW])
            nc.scalar.dma_start(par[:], par_d[:])
            nc.scalar.dma_start(eye[:], eye_d[:])
            nc.gpsimd.dma_start(mkl[:], mkl_d[:])
            nc.gpsimd.dma_start(mkr[:], mkr_d[:])
            nc.gpsimd.dma_start(decT[:], decT_d[:])

            # zero dconv overhang strips of p once
            for ct in range(4):
                nc.vector.memset(p[:, ct, 0:DOFF].bitcast(F32), 0.0)
                nc.vector.memset(p[:, ct, DOFF + NE:BW].bitcast(F32), 0.0)

            # warm the ACT table set early (parametric_relu+sigmoid+identity)
            nc.vector.memset(warm[:], 0.0)
            nc.scalar.activation(warm[:], warm[:], AF.Prelu, bias=0.0, scale=1.0, alpha=0.25)
            nc.scalar.activation(warm[:], warm[:], AF.Sigmoid, bias=0.0, scale=1.0)

            def mm_group(lhts, rhs_of, halves, evict_fn, name, mrows=128,
                         half_outer=False):
                """One output row-tile: kt-outer matmuls into per-half psum
                tiles (2 banks each), then per-half evictions."""
                tiles = [psp.tile([128, 1024], F32, tag="ps", name=f"{name}{hi}")
                         for hi in range(len(halves))]
                nk = len(lhts)
                order = ([(hi, ki) for hi in range(len(halves)) for ki in range(nk)]
                         if half_outer else
                         [(hi, ki) for ki in range(nk) for hi in range(len(halves))])
                for hi, ki in order:
                    hv = halves[hi]
                    for (po, do, w) in hv["segs"]:
                        nc.tensor.matmul(
                            tiles[hi][0:mrows, po:po + w], lhts[ki],
                            rhs_of(ki, do, w),
                            start=(ki == 0), stop=(ki == nk - 1),
                            skip_group_check=True,
                        )
                    if ki == nk - 1 and half_outer:
                        (po, do, w) = hv["ev"]
                        evict_fn(tiles[hi], po, do, w)
                if not half_outer:
                    for hi, hv in enumerate(halves):
                        (po, do, w) = hv["ev"]
                        evict_fn(tiles[hi], po, do, w)

            def emit_body():
                # ---- encoder: enc = encT.T @ win (K=20), evict with +enc_b ----
                for mt in range(2):
                    ebias = par[:, NL * PCOLS_PER_LAYER + mt: NL * PCOLS_PER_LAYER + mt + 1]
                    def enc_evict(ps, po, do, w, mt=mt, ebias=ebias):
                        nc.scalar.activation(
                            HI0[:, mt, DOFF + do:DOFF + do + w], ps[:, po:po + w],
                            AF.Identity, bias=ebias, scale=1.0)
                    mm_group(
                        [encT[:, mt * 128:(mt + 1) * 128]],
                        lambda ki, do, w: win[:, do:do + w],
                        HALVES0, enc_evict, f"enc{mt}")

                # ---- TCN ----
                hcur = HI0
                for b in range(BL):
                    halves = HALVES0 if b == 0 else HALVES1
                    resid = hcur
                    for l in range(L):
                        li = b * L + l
                        base = li * PCOLS_PER_LAYER
                        dil = 1 << l

                        w1t = lw.tile([128, 2, D], F32R, tag="w1t")
                        w2t = lw.tile([128, 4, E], F32R, tag="w2t")
                        dg = lw.tile([128, 12, 128], F32R, tag="dg")
                        nc.sync.dma_start(w1t[:, :, 0:256], w1T_d[li, :, :, 0:256])
                        nc.sync.dma_start(w1t[:, :, 256:D], w1T_d[li, :, :, 256:D])
                        nc.gpsimd.dma_start(w2t[:], w2T_d[li])
                        # diag matrices for taps: dg[:, ct*3+k, :]
                        for ct in range(4):
                            for k in range(3):
                                nc.vector.tensor_scalar_mul(
                                    dg[:, ct * 3 + k, :], eye[:],
                                    par[:, base + 8 + 4 * k + ct: base + 9 + 4 * k + ct],
                                )

                        # conv1 (E->D) + Prelu/BN eviction into p
                        for ct in range(4):
                            def ev1(ps, po, do, w, ct=ct):
                                nc.scalar.activation(
                                    p[:, ct, DOFF + do:DOFF + do + w], ps[:, po:po + w],
                                    AF.Prelu,
                                    bias=par[:, base + 4 + ct: base + 5 + ct],
                                    scale=par[:, base + ct: base + 1 + ct],
                                    alpha=par[:, base + 38: base + 39],
                                )
                            mm_group(
                                [w1t[:, kt, ct * 128:(ct + 1) * 128] for kt in range(2)],
                                lambda ki, do, w: hcur[:, ki, DOFF + do:DOFF + do + w],
                                halves, ev1, f"c1_{ct}_")
                            # zero-pad masks on the dconv input (per-core data),
                            # then fill tap-reachable pad cols with -C1 so the
                            # folded dconv bias is exact at true tensor edges
                            nc.vector.tensor_mul(
                                p[:, ct, 96:160], p[:, ct, 96:160], mkl[:])
                            nc.vector.tensor_scalar_add(
                                p[:, ct, 128:160], p[:, ct, 128:160],
                                par[:, base + 28 + ct: base + 29 + ct])
                            nc.vector.tensor_mul(
                                p[:, ct, 1760:1824], p[:, ct, 1760:1824], mkr[:])
                            nc.vector.tensor_scalar_add(
                                p[:, ct, 1760:1792], p[:, ct, 1760:1792],
                                par[:, base + 32 + ct: base + 33 + ct])

                        # depthwise dilated conv: outer taps as 2 diagonal
                        # matmuls on PE, center tap fused on DVE into PSUM,
                        # then Prelu/BN eviction into v
                        for ct in range(4):
                            def ev2(ps, po, do, w, ct=ct):
                                nc.vector.scalar_tensor_tensor(
                                    ps[:, po:po + w],
                                    p[:, ct, DOFF + do:DOFF + do + w],
                                    par[:, base + 12 + ct: base + 13 + ct],
                                    ps[:, po:po + w],
                                    op0=OP.mult, op1=OP.add,
                                )
                                nc.scalar.activation(
                                    v[:, ct, DOFF + do:DOFF + do + w], ps[:, po:po + w],
                                    AF.Prelu,
                                    bias=par[:, base + 24 + ct: base + 25 + ct],
                                    scale=par[:, base + 20 + ct: base + 21 + ct],
                                    alpha=par[:, base + 39: base + 40],
                                )
                            mm_group(
                                [dg[:, ct * 3 + k, :] for k in (0, 2)],
                                lambda ki, do, w, ct=ct: p[:, ct, DOFF + (2 * ki - 1) * dil + do:
                                                           DOFF + (2 * ki - 1) * dil + do + w],
                                halves, ev2, f"dc_{ct}_")

                        # conv2 (D->E) + h update
                        last = (l == L - 1)
                        hnext = (HI1 if b == 0 else hF) if last else hP
                        for ct2 in range(2):
                            eb = par[:, base + 36 + ct2: base + 37 + ct2]
                            def ev3(ps, po, do, w, ct2=ct2, eb=eb, last=last):
                                if last:
                                    nc.vector.scalar_tensor_tensor(
                                        hnext[:, ct2, DOFF + do:DOFF + do + w],
                                        ps[:, po:po + w], eb,
                                        resid[:, ct2, DOFF + do:DOFF + do + w],
                                        op0=OP.add, op1=OP.add,
                                    )
                                else:
                                    nc.vector.tensor_scalar_add(
                                        hnext[:, ct2, DOFF + do:DOFF + do + w],
                                        ps[:, po:po + w], eb)
                            mm_group(
                                [w2t[:, kt, ct2 * 128:(ct2 + 1) * 128] for kt in range(4)],
                                lambda ki, do, w: v[:, ki, DOFF + do:DOFF + do + w],
                                halves, ev3, f"c2_{ct2}_")
                        hcur = hnext

                # ---- mask + decoder (full range, HALVES0 layout) ----
                sig = p  # reuse
                mk = v
                for ct2 in range(2):
                    for (c0, c1) in ((0, 512), (512, 1024), (1024, 1536), (1536, NE)):
                        nc.scalar.activation(
                            sig[:, ct2, DOFF + c0:DOFF + c1], hF[:, ct2, DOFF + c0:DOFF + c1],
                            AF.Sigmoid, bias=0.0, scale=1.0)
                        nc.vector.tensor_mul(
                            mk[:, ct2, DOFF + c0:DOFF + c1],
                            HI0[:, ct2, DOFF + c0:DOFF + c1],
                            sig[:, ct2, DOFF + c0:DOFF + c1])
                dsb = per.tile([20, NE], F32)
                def evd(ps, po, do, w):
                    nc.scalar.activation(
                        dsb[:, do:do + w], ps[0:20, po:po + w], AF.Copy)
                mm_group(
                    [decT[:, kt, :] for kt in range(2)],
                    lambda ki, do, w: mk[:, ki, DOFF + do:DOFF + do + w],
                    HALVES0, evd, "dec_", mrows=20, half_outer=True)
                # out[10m+r] = P1[r, m+MARG+2] + P2[r, m+MARG+1]  (host adds them)
                ys = 1024 - MARG - 2  # first-piece width aligned to dsb piece A
                nc.sync.dma_start(y1_d[:, 0:ys], dsb[0:10, MARG + 2:1024])
                nc.sync.dma_start(y2_d[:, 0:ys + 1], dsb[10:20, MARG + 1:1024])
                nc.sync.dma_start(y1_d[:, ys:NI], dsb[0:10, 1024:MARG + 2 + NI])
                nc.sync.dma_start(y2_d[:, ys + 1:NI], dsb[10:20, 1024:MARG + 1 + NI])


            if loop_k is None:
                emit_body()
            else:
                with tc.For_i(0, loop_k):
                    emit_body()

    _split_multi_waits(nc)
    return nc


def _host_prep(inputs):
    """Per-core in_maps + assembly metadata from full inputs."""
    f32 = np.float32
    x = np.asarray(inputs["x"], f32)
    enc_w = np.asarray(inputs["enc_w"], f32)
    enc_b = np.asarray(inputs["enc_b"], f32)
    w1 = np.asarray(inputs["w1"], f32)
    b1 = np.asarray(inputs["b1"], f32)
    a1 = np.asarray(inputs["a1"], f32)
    g1 = np.asarray(inputs["g1"], f32)
    be1 = np.asarray(inputs["be1"], f32)
    m1 = np.asarray(inputs["m1"], f32)
    v1 = np.asarray(inputs["v1"], f32)
    wd = np.asarray(inputs["wd"], f32)
    bd = np.asarray(inputs["bd"], f32)
    a2 = np.asarray(inputs["a2"], f32)
    g2 = np.asarray(inputs["g2"], f32)
    be2 = np.asarray(inputs["be2"], f32)
    m2 = np.asarray(inputs["m2"], f32)
    v2 = np.asarray(inputs["v2"], f32)
    w2 = np.asarray(inputs["w2"], f32)
    b2 = np.asarray(inputs["b2"], f32)
    dec_w = np.asarray(inputs["dec_w"], f32)
    dec_b = np.asarray(inputs["dec_b"], f32)

    eye = np.eye(128, dtype=f32)
    encT = np.ascontiguousarray(enc_w[:, 0, :].T)  # [FK, E]
    decT = np.zeros((128, 2, 20), f32)
    for kt in range(2):
        decT[:, kt, :] = dec_w[kt * 128:(kt + 1) * 128, 0, :]

    w1T = np.zeros((NL, 128, 2, D), f32)
    w2T = np.zeros((NL, 128, 4, E), f32)
    C1 = np.zeros((NL, D), np.float64)
    taps = np.zeros((NL, 3, D), np.float64)
    par_shared = np.zeros((128, NPCOL), f32)
    for b in range(BL):
        for l in range(L):
            li = b * L + l
            base = li * PCOLS_PER_LAYER
            w1bl = w1[b, l, :, :, 0].astype(np.float64)  # [D, E]
            w2bl = w2[b, l, :, :, 0].astype(np.float64)  # [E, D]
            for kt in range(2):
                w1T[li, :, kt, :] = w1bl.T[kt * 128:(kt + 1) * 128, :]
            for kt in range(4):
                w2T[li, :, kt, :] = w2bl.T[kt * 128:(kt + 1) * 128, :]
            s1 = g1[b, l] / np.sqrt(v1[b, l].astype(np.float64) + EPS)
            c1 = be1[b, l] - m1[b, l] * s1
            s2 = g2[b, l] / np.sqrt(v2[b, l].astype(np.float64) + EPS)
            c2 = be2[b, l] - m2[b, l] * s2
            C1[li] = c1
            w0, w1c, w2c = (wd[b, l, :, 0, k].astype(np.float64) for k in range(3))
            taps[li] = np.stack([w0, w1c, w2c])
            bias2p = bd[b, l] + c1 * (w0 + w1c + w2c)
            b2pp = b2[b, l] + w2bl @ c2

            def col(idx, vals512):
                par_shared[:, base + idx:base + idx + 4] = np.asarray(
                    vals512, f32).reshape(4, 128).T
            col(0, s1)
            col(4, s1 * b1[b, l])
            for k in range(3):
                col(8 + 4 * k, taps[li, k])
            col(20, s2)
            col(24, s2 * bias2p)
            # 28..35: fixL/fixR are per-core (filled later)
            par_shared[:, base + 36:base + 38] = np.asarray(
                b2pp, f32).reshape(2, 128).T
            par_shared[:, base + 38] = a1[b, l]
            par_shared[:, base + 39] = a2[b, l]
    par_shared[:, NL * PCOLS_PER_LAYER:NL * PCOLS_PER_LAYER + 2] = \
        enc_b.reshape(2, 128).T

    in_maps = []
    ones64 = np.ones((128, 64), f32)
    zeros64 = np.zeros((128, 64), f32)
    for core in range(NCORES):
        bb, q = divmod(core, QP)
        xbase = 16010 * q - 1300
        xw = np.zeros(XW_LEN, f32)
        lo, hi = max(0, xbase), min(T, xbase + XW_LEN)
        if hi > lo:
            xw[lo - xbase:hi - xbase] = x[bb, 0, lo:hi]
        # im2col on host: win[k, j] = xw[10j + k]
        winm = np.lib.stride_tricks.as_strided(
            xw, shape=(1920, FK), strides=(40, 4)).T.copy()
        par = par_shared.copy()
        left, right = (q == 0), (q == QP - 1)
        for li in range(NL):
            base = li * PCOLS_PER_LAYER
            fixL = (-C1[li]) if left else np.zeros(D)
            fixR = (-C1[li]) if right else np.zeros(D)
            par[:, base + 28:base + 32] = np.asarray(fixL, f32).reshape(4, 128).T
            par[:, base + 32:base + 36] = np.asarray(fixR, f32).reshape(4, 128).T
        in_maps.append(dict(
            win=winm, eye=eye,
            maskL=(zeros64 if left else ones64),
            maskR=(zeros64 if right else ones64),
            params=par, encT=encT, decT=decT, w1T=w1T, w2T=w2T,
        ))
    return in_maps, float(dec_b[0])


def kernel(**inputs):
    global _built
    if _built is None:
        _built = build()
    nc = _built
    in_maps, decb = _host_prep(inputs)
    res = run_bass_kernel_spmd(nc, in_maps, core_ids=list(range(NCORES)))
    out = np.zeros((B, 1, T), np.float32)
    for core in range(NCORES):
        bb, q = divmod(core, QP)
        seg = (res.results[core]["y1"] + res.results[core]["y2"]).T.reshape(-1)
        t0 = q * NI * STR
        n = min(T - t0, NI * STR)
        out[bb, 0, t0:t0 + n] = seg[:n] + decb
    return out



# revision 8
# speedup vs baseline: 1.1419x; 1.1419x over previous
"""BitwiseTasNet Trainium2 kernel (fp8 DoubleRow edition).

Full (unsharded) inputs in, full output out. Internally: data-parallel over
batch x time across 8 NeuronCores (4 time-shards per batch item) with halo
margins so no inter-core communication is needed.

Precision plan (validated numerically, rel_l2 ~ 5.4e-3 vs 2e-2 gate):
 - conv1 / conv2 / dconv run in fp8e4m3 with DoubleRow perf mode (0.5
   cycles/row on PE, 2x the bf16 rate); the h stream, p and v activations
   are stored fp8.
 - encoder, decoder, and the mask multiply stay fp16 (fp8 there fails).

Engine split per TCN layer (W~1857 cols):
 - PE: conv1 (1 DR pass/ct), dconv (1 DR outer-tap pair + 1 plain center
   diag per ct, host-precomputed diagonal tiles), conv2 (2 DR per ct2).
 - ACT: prelu+BN evictions for ct 0-2 of conv1/dconv outputs.
 - DVE: ct3 evictions as 3-op chains (tensor_scalar from PSUM, x alpha,
   max), edge masks/fixups.
 - Pool (gpsimd): conv2 evictions (+bias, +residual) and one ev2 half.
"""
import sys

sys.path.insert(0, "/opt/trn_rl_repo")

import numpy as np

import concourse.bass as bass
import concourse.mybir as mybir
import concourse.tile as tile
from concourse.ap import AP
from concourse.bass_utils import run_bass_kernel_spmd

# Problem constants (hardcoded per contest rules).
B, T, E, D, BL, L, KT, FK, STR = 2, 64000, 256, 512, 2, 6, 3, 20, 10
EPS = 1e-5
TC = (T + 2 * FK - FK) // STR + 1  # 6403 encoder output cols
NCORES, QP = 8, 4  # 4 time-shards per batch item
NI = 1601          # interior cols per core (ceil(6403/4))
MARG = 128         # halo margin (2*63 receptive field + 2 for decoder)
NE = NI + 2 * MARG # 1857 computed cols (block 0 / encoder / decoder)
DOFF = 32          # side strip for dconv tap overhang (max dilation)
BW = 1984          # activation buffer width
XW_LEN = 19240
NL = BL * L
PCOLS_PER_LAYER = 40
NPCOL = NL * PCOLS_PER_LAYER + 8

# PSUM groups are half-width (2 banks, 4 slots) for pipeline depth. Each
# half: matmul segments (psum_off, data_off, width) and an eviction run.
# Matmul widths are 128-multiples at bank-aligned psum offsets.
# Block 1 only needs cols [63, 1794) (output valid on [126, 1731)).
HALVES0 = [
    dict(segs=[(0, 0, 512), (512, 512, 512)], ev=(0, 0, 1024)),
    dict(segs=[(0, 1024, 512), (512, 1536, 384)], ev=(0, 1024, 833)),
]
HALVES1 = [
    dict(segs=[(0, 63, 512), (512, 575, 512)], ev=(0, 63, 1024)),
    dict(segs=[(0, 1087, 512), (512, 1599, 256)], ev=(0, 1087, 707)),
]
NEW = 1920  # encoder window width (block-0 matmuls span [0, 1920))

F32 = mybir.dt.float32
F16 = mybir.dt.float16
F8 = mybir.dt.float8e4
AF = mybir.ActivationFunctionType
OP = mybir.AluOpType
DR = mybir.MatmulPerfMode.DoubleRow

_built = None  # cached (module is data-independent)


def _split_multi_waits(nc, max_waits=1):
    """This walrus build accepts only one sync-wait command per instruction;
    hoist extras into standalone NoOps on the same engine just before it."""
    for fn in nc.m.functions:
        for blk in fn.blocks:
            new_insts, ctr = [], 0
            for inst in blk.instructions:
                si = inst.sync_info
                if si is not None and len(si.on_wait) > max_waits:
                    extra = si.on_wait[:-max_waits]
                    si.on_wait = si.on_wait[-max_waits:]
                    for w in extra:
                        ctr += 1
                        new_insts.append(mybir.InstNoOp(
                            name=f"{inst.name}_hw{ctr}",
                            engine=inst.engine,
                            sync_info=mybir.SyncInfo(on_wait=[w], on_update=[]),
                            bass_nofuse=True,
                        ))
                new_insts.append(inst)
            blk.instructions = new_insts


def build(loop_k=None):
    """Build the (data-independent) bass module for one core."""
    nc = bass.Bass()

    win_d = nc.dram_tensor("win", [FK, NEW], F16, kind="ExternalInput")
    mkl_d = nc.dram_tensor("maskL", [128, 64], F8, kind="ExternalInput")
    mkr_d = nc.dram_tensor("maskR", [128, 64], F8, kind="ExternalInput")
    par_d = nc.dram_tensor("params", [128, NPCOL], F32, kind="ExternalInput")
    encT_d = nc.dram_tensor("encT", [FK, E], F16, kind="ExternalInput")
    decT_d = nc.dram_tensor("decT", [128, 2, 20], F16, kind="ExternalInput")
    w1T_d = nc.dram_tensor("w1T", [NL, 128, 2, D], F8, kind="ExternalInput")
    w2T_d = nc.dram_tensor("w2T", [NL, 128, 4, E], F8, kind="ExternalInput")
    dg_d = nc.dram_tensor("dg", [NL, 128, 12, 128], F8, kind="ExternalInput")
    y1_d = nc.dram_tensor("y1", [10, NI], F32, kind="ExternalOutput")
    y2_d = nc.dram_tensor("y2", [10, NI], F32, kind="ExternalOutput")

    with tile.TileContext(nc) as tc:
        with (
            tc.tile_pool(name="per", bufs=1) as per,
            tc.tile_pool(name="lw", bufs=3) as lw,
            tc.tile_pool(name="ps", bufs=4, space="PSUM") as psp,
        ):
            # ---- persistent tiles ----
            mkl = per.tile([128, 64], F8)
            mkr = per.tile([128, 64], F8)
            par = per.tile([128, NPCOL], F32)
            encT = per.tile([FK, E], F16)
            decT = per.tile([128, 2, 20], F16)
            win = per.tile([FK, NEW], F16)
            HI016 = per.tile([128, 2, BW], F16)  # enc (mask source, b0 resid)
            HI08 = per.tile([128, 2, BW], F8)    # enc fp8 (b0 conv1 input)
            HI1 = per.tile([128, 2, BW], F8)     # block1 input / resid
            hP = per.tile([128, 2, BW], F8)      # intra-block h scratch
            hF = per.tile([128, 2, BW], F8)      # final h
            p = per.tile([128, 4, BW], F8)       # prelu1 out (dconv input)
            v = per.tile([128, 4, BW], F8)       # prelu2 out (conv2 input)
            u1 = per.tile([128, BW], F16)        # DVE eviction scratch
            q16 = per.tile([128, BW], F16)       # DVE eviction scratch
            ms16 = per.tile([128, 2, BW], F16)   # masked enc (decoder rhs)
            warm = per.tile([128, 1], F32)
            dsb = per.tile([20, NE], F32)

            nc.scalar.dma_start(encT[:], encT_d[:])
            nc.sync.dma_start(win[:, 0:1024], win_d[:, 0:1024])
            nc.sync.dma_start(win[:, 1024:NEW], win_d[:, 1024:NEW])
            nc.scalar.dma_start(par[:], par_d[:])
            nc.gpsimd.dma_start(mkl[:], mkl_d[:])
            nc.gpsimd.dma_start(mkr[:], mkr_d[:])
            nc.gpsimd.dma_start(decT[:], decT_d[:])

            # zero dconv overhang strips of p once
            for ct in range(4):
                nc.vector.memset(p[:, ct, 0:DOFF], 0.0)
                nc.vector.memset(p[:, ct, DOFF + NE:BW], 0.0)

            # warm the ACT table set early (parametric_relu+sigmoid+identity)
            nc.vector.memset(warm[:], 0.0)
            nc.scalar.activation(warm[:], warm[:], AF.Prelu, bias=0.0, scale=1.0, alpha=0.25)
            nc.scalar.activation(warm[:], warm[:], AF.Sigmoid, bias=0.0, scale=1.0)

            pfull = p[:]
            PSTRIDE = pfull.ap[0][0]
            PBASE = pfull.offset

            def p_dr_rhs(ct, dil, do, w):
                # [128, 2, w] view of p[ct]: rows (p shifted -dil, p shifted +dil)
                off = PBASE + ct * BW + DOFF + do - dil
                return AP(tensor=pfull.tensor, offset=off,
                          ap=[[PSTRIDE, 128], [2 * dil, 2], [1, w]])

            def mm_group(passes, halves, evict_fn, name, mrows=128):
                """One output row-tile: per-half psum tiles; each pass is
                (lhsT, rhs_of(do, w), perf_mode), chained via start/stop."""
                tiles = [psp.tile([128, 1024], F32, tag="ps", name=f"{name}{hi}")
                         for hi in range(len(halves))]
                nk = len(passes)
                for hi, hv in enumerate(halves):
                    for ki, (lht, rhs_of, pm) in enumerate(passes):
                        for (po, do, w) in hv["segs"]:
                            nc.tensor.matmul(
                                tiles[hi][0:mrows, po:po + w], lht,
                                rhs_of(do, w),
                                start=(ki == 0), stop=(ki == nk - 1),
                                perf_mode=pm,
                                skip_group_check=True,
                            )
                for hi, hv in enumerate(halves):
                    (po, do, w) = hv["ev"]
                    evict_fn(tiles[hi], po, do, w, hi)

            def dve_prelu_chain(dst, ps, po, do, w, sc, bc, ac, tail):
                """dst[cols] = prelu(scale*ps + bias) via 3 tensor ops.
                PSUM read stays on DVE (gpsimd cannot access PSUM); the
                SBUF-only tail ops run on `tail` (DVE or Pool)."""
                u = u1[:, DOFF + do:DOFF + do + w]
                q = q16[:, DOFF + do:DOFF + do + w]
                nc.vector.tensor_scalar(u, ps[:, po:po + w], sc, bc,
                                        op0=OP.mult, op1=OP.add)
                tail.tensor_scalar(q, u, ac, None, op0=OP.mult)
                tail.tensor_tensor(dst, u, q, op=OP.max)

            def emit_body():
                # ---- encoder: enc = encT.T @ win (K=20) -> fp16 + fp8 copies
                for mt in range(2):
                    ebias = par[:, NL * PCOLS_PER_LAYER + mt: NL * PCOLS_PER_LAYER + mt + 1]
                    def enc_evict(ps, po, do, w, hi, mt=mt, ebias=ebias):
                        nc.scalar.activation(
                            HI016[:, mt, DOFF + do:DOFF + do + w], ps[:, po:po + w],
                            AF.Identity, bias=ebias, scale=1.0)
                        nc.vector.tensor_scalar(
                            HI08[:, mt, DOFF + do:DOFF + do + w], ps[:, po:po + w],
                            1.0, ebias, op0=OP.mult, op1=OP.add)
                    mm_group(
                        [(encT[:, mt * 128:(mt + 1) * 128],
                          lambda do, w: win[:, do:do + w], None)],
                        HALVES0, enc_evict, f"enc{mt}")

                # ---- TCN ----
                hcur = HI08
                for b in range(BL):
                    halves = HALVES0 if b == 0 else HALVES1
                    resid = HI016 if b == 0 else HI1
                    for l in range(L):
                        li = b * L + l
                        base = li * PCOLS_PER_LAYER
                        dil = 1 << l

                        w1t = lw.tile([128, 2, D], F8, tag="w1t")
                        w2t = lw.tile([128, 4, E], F8, tag="w2t")
                        dgt = lw.tile([128, 12, 128], F8, tag="dgt")
                        nc.sync.dma_start(w1t[:], w1T_d[li])
                        nc.sync.dma_start(w2t[:], w2T_d[li])
                        nc.gpsimd.dma_start(dgt[:], dg_d[li])

                        # conv1 (E->D, one DR pass) + Prelu/BN eviction into p
                        for ct in range(4):
                            if ct < 3:
                                def ev1(ps, po, do, w, hi, ct=ct):
                                    nc.scalar.activation(
                                        p[:, ct, DOFF + do:DOFF + do + w], ps[:, po:po + w],
                                        AF.Prelu,
                                        bias=par[:, base + 4 + ct: base + 5 + ct],
                                        scale=par[:, base + ct: base + 1 + ct],
                                        alpha=par[:, base + 38: base + 39],
                                    )
                            else:
                                def ev1(ps, po, do, w, hi, ct=ct):
                                    dve_prelu_chain(
                                        p[:, ct, DOFF + do:DOFF + do + w],
                                        ps, po, do, w,
                                        par[:, base + 3: base + 4],
                                        par[:, base + 7: base + 8],
                                        par[:, base + 38: base + 39],
                                        tail=nc.vector)
                            mm_group(
                                [(w1t[:, :, ct * 128:(ct + 1) * 128],
                                  lambda do, w: hcur[:, 0:2, DOFF + do:DOFF + do + w], DR)],
                                halves, ev1, f"c1_{ct}_")
                            # zero-pad masks on the dconv input (per-core data),
                            # then fill tap-reachable pad cols with -C1 so the
                            # folded dconv bias is exact at true tensor edges
                            nc.vector.tensor_mul(
                                p[:, ct, 96:160], p[:, ct, 96:160], mkl[:])
                            nc.vector.tensor_scalar_add(
                                p[:, ct, 128:160], p[:, ct, 128:160],
                                par[:, base + 28 + ct: base + 29 + ct])
                            nc.vector.tensor_mul(
                                p[:, ct, 1760:1824], p[:, ct, 1760:1824], mkr[:])
                            nc.vector.tensor_scalar_add(
                                p[:, ct, 1760:1792], p[:, ct, 1760:1792],
                                par[:, base + 32 + ct: base + 33 + ct])

                        # depthwise dilated conv: DR pair (outer taps) + plain
                        # center diag on PE, then Prelu/BN eviction into v
                        for ct in range(4):
                            if ct < 3:
                                def ev2(ps, po, do, w, hi, ct=ct):
                                    nc.scalar.activation(
                                        v[:, ct, DOFF + do:DOFF + do + w], ps[:, po:po + w],
                                        AF.Prelu,
                                        bias=par[:, base + 24 + ct: base + 25 + ct],
                                        scale=par[:, base + 20 + ct: base + 21 + ct],
                                        alpha=par[:, base + 39: base + 40],
                                    )
                            else:
                                def ev2(ps, po, do, w, hi, ct=ct):
                                    dve_prelu_chain(
                                        v[:, ct, DOFF + do:DOFF + do + w],
                                        ps, po, do, w,
                                        par[:, base + 23: base + 24],
                                        par[:, base + 27: base + 28],
                                        par[:, base + 39: base + 40],
                                        tail=nc.vector)
                            mm_group(
                                [(dgt[:, ct * 3:ct * 3 + 2, :],
                                  lambda do, w, ct=ct: p_dr_rhs(ct, dil, do, w), DR),
                                 (dgt[:, ct * 3 + 2, :],
                                  lambda do, w, ct=ct: p[:, ct, DOFF + do:DOFF + do + w], None)],
                                halves, ev2, f"dc_{ct}_")

                        # conv2 (D->E, two DR passes) + h update on Pool
                        last = (l == L - 1)
                        hnext = (HI1 if b == 0 else hF) if last else hP
                        for ct2 in range(2):
                            eb = par[:, base + 36 + ct2: base + 37 + ct2]
                            def ev3(ps, po, do, w, hi, ct2=ct2, eb=eb, last=last):
                                if last:
                                    nc.vector.scalar_tensor_tensor(
                                        hnext[:, ct2, DOFF + do:DOFF + do + w],
                                        ps[:, po:po + w], eb,
                                        resid[:, ct2, DOFF + do:DOFF + do + w],
                                        op0=OP.add, op1=OP.add,
                                    )
                                else:
                                    nc.vector.tensor_scalar_add(
                                        hnext[:, ct2, DOFF + do:DOFF + do + w],
                                        ps[:, po:po + w], eb)
                            mm_group(
                                [(w2t[:, 0:2, ct2 * 128:(ct2 + 1) * 128],
                                  lambda do, w: v[:, 0:2, DOFF + do:DOFF + do + w], DR),
                                 (w2t[:, 2:4, ct2 * 128:(ct2 + 1) * 128],
                                  lambda do, w: v[:, 2:4, DOFF + do:DOFF + do + w], DR)],
                                halves, ev3, f"c2_{ct2}_")
                        hcur = hnext

                # ---- mask + decoder (full range, HALVES0 layout) ----
                for ct2 in range(2):
                    sg = u1 if ct2 == 0 else q16
                    for (c0, c1) in ((0, 1024), (1024, NE)):
                        nc.scalar.activation(
                            sg[:, DOFF + c0:DOFF + c1], hF[:, ct2, DOFF + c0:DOFF + c1],
                            AF.Sigmoid, bias=0.0, scale=1.0)
                        nc.vector.tensor_mul(
                            ms16[:, ct2, DOFF + c0:DOFF + c1],
                            HI016[:, ct2, DOFF + c0:DOFF + c1],
                            sg[:, DOFF + c0:DOFF + c1])
                def evd(ps, po, do, w, hi):
                    nc.scalar.activation(
                        dsb[:, do:do + w], ps[0:20, po:po + w], AF.Copy)
                mm_group(
                    [(decT[:, 0, :],
                      lambda do, w: ms16[:, 0, DOFF + do:DOFF + do + w], None),
                     (decT[:, 1, :],
                      lambda do, w: ms16[:, 1, DOFF + do:DOFF + do + w], None)],
                    HALVES0, evd, "dec_", mrows=20)
                # out[10m+r] = P1[r, m+MARG+2] + P2[r, m+MARG+1]  (host adds them)
                ys = 1024 - MARG - 2  # first-piece width aligned to dsb piece A
                nc.sync.dma_start(y1_d[:, 0:ys], dsb[0:10, MARG + 2:1024])
                nc.sync.dma_start(y2_d[:, 0:ys + 1], dsb[10:20, MARG + 1:1024])
                nc.sync.dma_start(y1_d[:, ys:NI], dsb[0:10, 1024:MARG + 2 + NI])
                nc.sync.dma_start(y2_d[:, ys + 1:NI], dsb[10:20, 1024:MARG + 1 + NI])

            if loop_k is None:
                emit_body()
            else:
                with tc.For_i(0, loop_k):
                    emit_body()

    _split_multi_waits(nc)
    return nc


def _host_prep(inputs):
    """Per-core in_maps + assembly metadata from full inputs."""
    f32 = np.float32
    f16 = mybir.dt.np(F16)
    f8 = mybir.dt.np(F8)
    x = np.asarray(inputs["x"], f32)
    enc_w = np.asarray(inputs["enc_w"], f32)
    enc_b = np.asarray(inputs["enc_b"], f32)
    w1 = np.asarray(inputs["w1"], f32)
    b1 = np.asarray(inputs["b1"], f32)
    a1 = np.asarray(inputs["a1"], f32)
    g1 = np.asarray(inputs["g1"], f32)
    be1 = np.asarray(inputs["be1"], f32)
    m1 = np.asarray(inputs["m1"], f32)
    v1 = np.asarray(inputs["v1"], f32)
    wd = np.asarray(inputs["wd"], f32)
    bd = np.asarray(inputs["bd"], f32)
    a2 = np.asarray(inputs["a2"], f32)
    g2 = np.asarray(inputs["g2"], f32)
    be2 = np.asarray(inputs["be2"], f32)
    m2 = np.asarray(inputs["m2"], f32)
    v2 = np.asarray(inputs["v2"], f32)
    w2 = np.asarray(inputs["w2"], f32)
    b2 = np.asarray(inputs["b2"], f32)
    dec_w = np.asarray(inputs["dec_w"], f32)
    dec_b = np.asarray(inputs["dec_b"], f32)

    encT = np.ascontiguousarray(enc_w[:, 0, :].T).astype(f16)  # [FK, E]
    decT = np.zeros((128, 2, 20), f16)
    for kt in range(2):
        decT[:, kt, :] = dec_w[kt * 128:(kt + 1) * 128, 0, :].astype(f16)

    w1T = np.zeros((NL, 128, 2, D), f8)
    w2T = np.zeros((NL, 128, 4, E), f8)
    dg = np.zeros((NL, 128, 12, 128), f8)
    C1 = np.zeros((NL, D), np.float64)
    par_shared = np.zeros((128, NPCOL), f32)
    idx = np.arange(128)
    for b in range(BL):
        for l in range(L):
            li = b * L + l
            base = li * PCOLS_PER_LAYER
            w1bl = w1[b, l, :, :, 0].astype(np.float64)  # [D, E]
            w2bl = w2[b, l, :, :, 0].astype(np.float64)  # [E, D]
            for kt in range(2):
                w1T[li, :, kt, :] = w1bl.T[kt * 128:(kt + 1) * 128, :].astype(f8)
            for kt in range(4):
                w2T[li, :, kt, :] = w2bl.T[kt * 128:(kt + 1) * 128, :].astype(f8)
            s1 = g1[b, l] / np.sqrt(v1[b, l].astype(np.float64) + EPS)
            c1 = be1[b, l] - m1[b, l] * s1
            s2 = g2[b, l] / np.sqrt(v2[b, l].astype(np.float64) + EPS)
            c2 = be2[b, l] - m2[b, l] * s2
            C1[li] = c1
            # taps quantized to fp8 (used as diag matmul weights on-chip)
            tq = [wd[b, l, :, 0, k].astype(f8).astype(np.float64) for k in range(3)]
            # diag tiles: slots ct*3+0 = tap0, +1 = tap2 (DR pair), +2 = tap1
            for ct in range(4):
                sl = slice(ct * 128, (ct + 1) * 128)
                dg[li, idx, ct * 3 + 0, idx] = tq[0][sl].astype(f8)
                dg[li, idx, ct * 3 + 1, idx] = tq[2][sl].astype(f8)
                dg[li, idx, ct * 3 + 2, idx] = tq[1][sl].astype(f8)
            bias2p = bd[b, l] + c1 * (tq[0] + tq[1] + tq[2])
            w2q = w2bl.astype(f8).astype(np.float64)
            b2pp = b2[b, l] + w2q @ c2

            def col(idx2, vals512):
                par_shared[:, base + idx2:base + idx2 + 4] = np.asarray(
                    vals512, f32).reshape(4, 128).T
            col(0, s1)
            col(4, s1 * b1[b, l])
            col(20, s2)
            col(24, s2 * bias2p)
            # 28..35: fixL/fixR are per-core (filled later)
            par_shared[:, base + 36:base + 38] = np.asarray(
                b2pp, f32).reshape(2, 128).T
            par_shared[:, base + 38] = a1[b, l]
            par_shared[:, base + 39] = a2[b, l]
    par_shared[:, NL * PCOLS_PER_LAYER:NL * PCOLS_PER_LAYER + 2] = \
        enc_b.reshape(2, 128).T

    in_maps = []
    ones64 = np.ones((128, 64), f8)
    zeros64 = np.zeros((128, 64), f8)
    for core in range(NCORES):
        bb, q = divmod(core, QP)
        xbase = 16010 * q - 1300
        xw = np.zeros(XW_LEN, f32)
        lo, hi = max(0, xbase), min(T, xbase + XW_LEN)
        if hi > lo:
            xw[lo - xbase:hi - xbase] = x[bb, 0, lo:hi]
        # im2col on host: win[k, j] = xw[10j + k]
        winm = np.lib.stride_tricks.as_strided(
            xw, shape=(1920, FK), strides=(40, 4)).T.astype(f16)
        par = par_shared.copy()
        left, right = (q == 0), (q == QP - 1)
        for li in range(NL):
            base = li * PCOLS_PER_LAYER
            fixL = (-C1[li]) if left else np.zeros(D)
            fixR = (-C1[li]) if right else np.zeros(D)
            par[:, base + 28:base + 32] = np.asarray(fixL, f32).reshape(4, 128).T
            par[:, base + 32:base + 36] = np.asarray(fixR, f32).reshape(4, 128).T
        in_maps.append(dict(
            win=winm,
            maskL=(zeros64 if left else ones64),
            maskR=(zeros64 if right else ones64),
            params=par, encT=encT, decT=decT, w1T=w1T, w2T=w2T, dg=dg,
        ))
    return in_maps, float(dec_b[0])


def kernel(**inputs):
    global _built
    if _built is None:
        _built = build()
    nc = _built
    in_maps, decb = _host_prep(inputs)
    res = run_bass_kernel_spmd(nc, in_maps, core_ids=list(range(NCORES)))
    out = np.zeros((B, 1, T), np.float32)
    for core in range(NCORES):
        bb, q = divmod(core, QP)
        seg = (res.results[core]["y1"] + res.results[core]["y2"]).T.reshape(-1)
        t0 = q * NI * STR
        n = min(T - t0, NI * STR)
        out[bb, 0, t0:t0 + n] = seg[:n] + decb
    return out
